# revision 4
# baseline (speedup 1.0000x reference)
"""Trainium2 Bass kernel for a dense transformer block (QKV+gate proj, RoPE,
QK-RMSNorm, causal SDPA, output-RMSNorm + SiLU gate, output projection).

Sharding: tensor-parallel over heads across 8 NeuronCores (2 heads/core).
Wq/Wk/Wv/Wg split column-wise; attention fully local per core; per-(batch,
head) attention outputs are AllGathered in 4 chunks (overlapped with
compute) and the output projection uses Wo column-split (each core emits a
256-column slice of the output).

v2 design notes:
- Projection sweeps use 1024-token chunks, k-outer loops (weight reuse
  across the two 512-halves), [128,1024] psum tiles (4 tags x 2 banks).
- All RMS partition-sums go through gpsimd.partition_all_reduce (no
  TensorE `ones` matmuls, no extra psum banks, no PE stalls on evac).
- The softmax denominator is never computed: rms_norm(y/s) is evaluated
  as ystash * rsqrt(sum_d(ystash^2)/D) (the eps*s^2 term is negligible).
- ScalarE activation-table thrash eliminated: sweep A uses only
  {copy, abs_reciprocal_sqrt}; sweep B only {copy, silu}; attention only
  {exp, ln} (rsqrt computed as exp(-0.5*ln(x))). 3 table loads total.
- silu(gate) computed in sweep B's evacuation and kept in SBUF.
- V kept d-major after sweep B; transposed on TensorE in each attention
  pair's prologue (PE has slack there; psum shared with the PV tag).
- AllGather split per (batch, head-chunk): 4 collectives that overlap
  attention; the final projection is interleaved into the attention
  instruction stream once its AG chunk has landed.
"""

import os
import sys

for _p in ("/opt/trn_rl_repo", "/root/.axon_site/_ro/trn_rl_repo"):
    if os.path.isdir(_p) and _p not in sys.path:
        sys.path.insert(0, _p)

import numpy as np

import concourse.bass_isa as bass_isa
import concourse.mybir as mybir
from concourse import bacc
from concourse.bass_utils import run_bass_kernel_spmd
from concourse.tile import TileContext

B, T, HID = 2, 2048, 2048
H, D = 16, 128
NCORES = 8
HC = H // NCORES          # heads per core = 2
DC = HC * D               # 256 head-dims per core
BT = B * T                # 4096 tokens
KT = HID // 128           # 16 contraction tiles
NCH = BT // 1024          # 4 projection chunks of 1024 tokens
EPS = 1e-5
SCALE = 1.0 / float(np.sqrt(D))
NEG = -3.0e38

F32 = mybir.dt.float32
BF16 = mybir.dt.bfloat16
AF = mybir.ActivationFunctionType
ALU = mybir.AluOpType
RED_ADD = bass_isa.ReduceOp.add

MMDT = BF16

LAST_EXEC_TIME_NS = None
_CACHED_NC = None


def _proj_sweep(nc, tc, xT, w_aps, posts):
    """One sweep over x computing 2 matrices (4 head-groups of 128) with
    1024-token chunks, k-outer (2 matmuls per weight load), [128,1024] psum
    per tag.  posts[mi](m, chunk, ps, tpool) evacuates one tag off the PE
    path (no TensorE work allowed)."""
    with tc.tile_pool(name="sweep_w", bufs=1) as wpool, \
         tc.tile_pool(name="sweep_x", bufs=4) as xpool, \
         tc.tile_pool(name="sweep_ps", bufs=1, space="PSUM") as pps, \
         tc.tile_pool(name="sweep_t", bufs=2) as tpool:
        wsb = []
        for mi, w_ap in enumerate(w_aps):
            w_t = wpool.tile([128, KT, DC], MMDT, tag=f"w{mi}", name=f"w{mi}")
            nc.sync.dma_start(out=w_t, in_=w_ap)
            wsb.append(w_t)
        for ch in range(NCH):
            ps = {}
            for mi in range(2):
                for m in range(HC):
                    ps[(mi, m)] = pps.tile([128, 1024], F32,
                                           tag=f"pp{mi}{m}", name=f"pp{mi}{m}")
            for k in range(KT):
                xk = xpool.tile([128, 1024], MMDT, tag="xk", name="xk")
                nc.sync.dma_start(out=xk, in_=xT[k, ch])
                for mi in range(2):
                    for m in range(HC):
                        lhsT = wsb[mi][:, k, m * 128:(m + 1) * 128]
                        for h in range(2):
                            hsl = slice(h * 512, (h + 1) * 512)
                            nc.tensor.matmul(
                                ps[(mi, m)][:, hsl], lhsT, xk[:, hsl],
                                start=(k == 0), stop=(k == KT - 1))
            for mi in range(2):
                for m in range(HC):
                    posts[mi](m, ch, ps[(mi, m)], tpool)


def _build_nc():
    nc = bacc.Bacc("TRN2", target_bir_lowering=False, debug=False,
                   num_devices=NCORES)

    xT = nc.dram_tensor("xT", [KT, NCH, 128, 1024], MMDT,
                        kind="ExternalInput").ap()
    wq = nc.dram_tensor("wq", [128, KT, DC], MMDT, kind="ExternalInput").ap()
    wk = nc.dram_tensor("wk", [128, KT, DC], MMDT, kind="ExternalInput").ap()
    wv = nc.dram_tensor("wv", [128, KT, DC], MMDT, kind="ExternalInput").ap()
    wg = nc.dram_tensor("wg", [128, KT, DC], MMDT, kind="ExternalInput").ap()
    wo = nc.dram_tensor("wo", [128, KT, DC], MMDT, kind="ExternalInput").ap()
    cos2 = nc.dram_tensor("cos2", [128, BT], F32, kind="ExternalInput").ap()
    sin2 = nc.dram_tensor("sin2", [128, BT], F32, kind="ExternalInput").ap()
    negm = nc.dram_tensor("negm", [128, 128], F32, kind="ExternalInput").ap()
    ident_in = nc.dram_tensor("ident_in", [128, 128], F32,
                              kind="ExternalInput").ap()
    qrw = nc.dram_tensor("qrw", [128, 1], F32, kind="ExternalInput").ap()
    krw = nc.dram_tensor("krw", [128, 1], F32, kind="ExternalInput").ap()

    outT = nc.dram_tensor("outT", [DC, BT], F32, kind="ExternalOutput").ap()
    ag_in = {}
    yall = {}
    for b in range(B):
        for m in range(HC):
            ag_in[(b, m)] = nc.dram_tensor(f"agin{b}{m}", [128, T],
                                           MMDT).ap()
            yall[(b, m)] = nc.dram_tensor(f"yall{b}{m}", [NCORES * 128, T],
                                          MMDT, addr_space="Shared").ap()

    with TileContext(nc) as tc:
      with tc.tile_pool(name="const", bufs=1) as const, \
           tc.tile_pool(name="persist", bufs=1) as persist:
        epsb = const.tile([128, 1], F32)
        nc.vector.memset(epsb, EPS)
        zerob = const.tile([128, 1], F32)
        nc.vector.memset(zerob, 0.0)

        # persistent per-head activations
        qTf = [persist.tile([128, BT], MMDT, tag=f"qTf{m}", name=f"qTf{m}")
               for m in range(HC)]
        kTf = [persist.tile([128, BT], MMDT, tag=f"kTf{m}", name=f"kTf{m}")
               for m in range(HC)]
        # v kept d-major in f32 (transposed per attention pair)
        v_dmaj = [persist.tile([128, BT], F32, tag=f"v{m}", name=f"v{m}")
                  for m in range(HC)]
        sg_sb = [persist.tile([128, BT], MMDT, tag=f"sg{m}", name=f"sg{m}")
                 for m in range(HC)]

        # ---------------- sweep A: q, k (rope + rms fused) ----------------
        with tc.tile_pool(name="ropec", bufs=1) as rp:
            cos_sb = rp.tile([128, BT], F32)
            sin_sb = rp.tile([128, BT], F32)
            qrw_sb = rp.tile([128, 1], F32)
            krw_sb = rp.tile([128, 1], F32)
            rope_loaded = [False]

            def load_rope_consts():
                # deferred so the first x/w DMAs win the queue
                nc.sync.dma_start(out=cos_sb, in_=cos2)
                nc.sync.dma_start(out=sin_sb, in_=sin2)
                nc.sync.dma_start(out=qrw_sb, in_=qrw)
                nc.sync.dma_start(out=krw_sb, in_=krw)

            def make_qk_post(dest, w_scalar):
                def post(m, ch, ps, tpool):
                    if not rope_loaded[0]:
                        rope_loaded[0] = True
                        load_rope_consts()
                    c0, c1 = ch * 1024, (ch + 1) * 1024
                    stage = tpool.tile([128, 1024], F32, tag="stage",
                                       name="stage", bufs=3)
                    nc.scalar.copy(stage, ps)
                    # rms sum of squares over head-dim (partitions)
                    sq = tpool.tile([128, 1024], MMDT, tag="sq", name="sq")
                    nc.vector.tensor_mul(sq, stage, stage)
                    ssb = tpool.tile([128, 1024], F32, tag="ssb", name="ssb")
                    nc.gpsimd.partition_all_reduce(ssb, sq, 128, RED_ADD)
                    fac = tpool.tile([128, 1024], F32, tag="fac", name="fac")
                    nc.scalar.activation(out=fac, in_=ssb,
                                         func=AF.Abs_reciprocal_sqrt,
                                         scale=1.0 / float(D), bias=epsb)
                    # rope: sw = swap-halves(stage); sin_sb has rows 0-63
                    # pre-negated so ro = stage*cos + sw*sin in one add
                    sw = tpool.tile([128, 1024], F32, tag="sw", name="sw")
                    nc.sync.dma_start(out=sw[0:64, :], in_=stage[64:128, :])
                    nc.sync.dma_start(out=sw[64:128, :], in_=stage[0:64, :])
                    u = tpool.tile([128, 1024], F32, tag="u", name="u")
                    nc.vector.tensor_mul(u, stage, cos_sb[:, c0:c1])
                    w = tpool.tile([128, 1024], F32, tag="w", name="w")
                    nc.gpsimd.tensor_mul(w, sw, sin_sb[:, c0:c1])
                    ro = tpool.tile([128, 1024], F32, tag="ro", name="ro")
                    nc.vector.tensor_add(ro, u, w)
                    # dest = (ro * w[d]) * fac
                    nc.vector.scalar_tensor_tensor(
                        out=dest[m][:, c0:c1], in0=ro, scalar=w_scalar,
                        in1=fac, op0=ALU.mult, op1=ALU.mult)
                return post

            _proj_sweep(nc, tc, xT, [wq, wk],
                        [make_qk_post(qTf, qrw_sb),
                         make_qk_post(kTf, krw_sb)])

        # ---------------- sweep B: v (d-major copy), gate (silu) ----------
        def v_post(m, ch, ps, tpool):
            nc.scalar.copy(v_dmaj[m][:, ch * 1024:(ch + 1) * 1024], ps)

        def g_post(m, ch, ps, tpool):
            nc.scalar.activation(
                out=sg_sb[m][:, ch * 1024:(ch + 1) * 1024], in_=ps,
                func=AF.Silu)

        _proj_sweep(nc, tc, xT, [wv, wg], [v_post, g_post])

        # ---------------- attention + chunked AG + final ------------------
        with tc.tile_pool(name="at_ps", bufs=1, space="PSUM") as pps2, \
             tc.tile_pool(name="fin_ps", bufs=1, space="PSUM") as fpps, \
             tc.tile_pool(name="at_slab", bufs=1) as slab, \
             tc.tile_pool(name="at_t", bufs=2) as tpool, \
             tc.tile_pool(name="at_bh", bufs=1) as bhpool, \
             tc.tile_pool(name="fin_w", bufs=1) as fwpool, \
             tc.tile_pool(name="fin_y", bufs=1) as fypool, \
             tc.tile_pool(name="fin_o", bufs=2) as fopool, \
             tc.tile_pool(name="at_c", bufs=1) as acp:
            negm_sb = acp.tile([128, 128], F32)
            nc.sync.dma_start(out=negm_sb, in_=negm)
            ident = acp.tile([128, 128], F32)
            nc.sync.dma_start(out=ident, in_=ident_in)
            wo_sb = fwpool.tile([128, KT, DC], MMDT, tag="wo")
            nc.sync.dma_start(out=wo_sb, in_=wo)

            def emit_pair(b, m, fillers):
                """Attention for batch b, head-chunk m. fillers: dict
                qb -> closure emitted after that qb's PV block."""
                t0 = b * T
                # prologue: transpose this pair's V to token-major
                vT = slab.tile([128, 16, 128], MMDT, tag="vT", name="vT",
                               bufs=2)
                for s in range(4):
                    tp = pps2.tile([128, 512], F32, tag="yt", name="tp",
                                   bufs=2)
                    for j in range(4):
                        i = s * 4 + j
                        nc.tensor.transpose(
                            tp[:, j * 128:(j + 1) * 128],
                            v_dmaj[m][:, t0 + i * 128:t0 + (i + 1) * 128],
                            ident)
                    nc.vector.tensor_copy(vT[:, s * 4:(s + 1) * 4, :], tp)

                ystash = bhpool.tile([128, T], F32, tag="bhC", name="ystash")
                ssy = bhpool.tile([128, T], F32, tag="bhS", name="ssy")
                for qb in range(T // 512):
                    nk = 4 * (qb + 1)
                    es = []
                    for i2 in range(0, nk, 2):
                        stp = pps2.tile([128, 1024], F32, tag="st",
                                        name="stp", bufs=2)
                        e = slab.tile([128, 1024], MMDT, tag=f"es{i2 // 2}",
                                      name=f"es{i2 // 2}")
                        for j in range(2):
                            i = i2 + j
                            sl = slice(j * 512, (j + 1) * 512)
                            nc.tensor.matmul(
                                stp[:, sl],
                                kTf[m][:, t0 + i * 128:t0 + (i + 1) * 128],
                                qTf[m][:, t0 + qb * 512:t0 + (qb + 1) * 512],
                                start=True, stop=True)
                            q_off = i * 128 - qb * 512
                            if q_off >= 0:
                                nc.vector.tensor_add(
                                    stp[:, j * 512 + q_off:
                                        j * 512 + q_off + 128],
                                    stp[:, j * 512 + q_off:
                                        j * 512 + q_off + 128],
                                    negm_sb)
                                if j == 0 and q_off > 0:
                                    nc.gpsimd.memset(e[:, 0:q_off], 0.0)
                                if j == 1 and q_off > 0:
                                    nc.vector.memset(
                                        stp[:, 512:512 + q_off], NEG)
                        q_off0 = i2 * 128 - qb * 512
                        lo = max(0, q_off0)
                        nc.scalar.activation(
                            out=e[:, lo:], in_=stp[:, lo:],
                            func=AF.Exp, scale=SCALE)
                        es.append(e)
                    ytp = pps2.tile([128, 512], F32, tag="yt", name="ytp",
                                    bufs=2)
                    for i2 in range(0, nk, 2):
                        for j in range(2):
                            i = i2 + j
                            sl = slice(j * 512, (j + 1) * 512)
                            nc.tensor.matmul(
                                ytp, vT[:, i, :], es[i2 // 2][:, sl],
                                start=(i == 0), stop=(i == nk - 1))
                    qsl = slice(qb * 512, (qb + 1) * 512)
                    nc.vector.tensor_copy(ystash[:, qsl], ytp)
                    sqy = tpool.tile([128, 512], MMDT, tag="sqy", name="sqy")
                    nc.vector.tensor_mul(sqy, ystash[:, qsl], ystash[:, qsl])
                    nc.gpsimd.partition_all_reduce(
                        ssy[:, qsl], sqy, 128, RED_ADD)
                    if qb in fillers:
                        fillers[qb]()
                # tail: fb2 = rsqrt(ssy/D) via exp(-0.5*ln); yf = ystash*fb2*sg
                lnv = bhpool.tile([128, T], F32, tag="bhB", name="lnv")
                nc.scalar.activation(out=lnv, in_=ssy, func=AF.Ln,
                                     scale=1.0 / float(D), bias=zerob)
                fb2 = bhpool.tile([128, T], F32, tag="bhS", name="fb2")
                nc.scalar.activation(out=fb2, in_=lnv, func=AF.Exp,
                                     scale=-0.5, bias=zerob)
                yf1 = bhpool.tile([128, T], F32, tag="bhB", name="yf1")
                nc.vector.tensor_mul(yf1, ystash, fb2)
                yf = bhpool.tile([128, T], MMDT, tag="bhC", name="yf")
                nc.vector.tensor_mul(yf, yf1, sg_sb[m][:, t0:t0 + T])
                nc.sync.dma_start(out=ag_in[(b, m)], in_=yf)
                nc.gpsimd.collective_compute(
                    "AllGather", ALU.bypass,
                    ins=[ag_in[(b, m)]], outs=[yall[(b, m)]],
                    replica_groups=[list(range(NCORES))],
                )

            def emit_final_tsp(b, tsp):
                """Final projection for 1024 tokens (2 ts-slabs of 512) of
                batch b; contracts all 16 kd tiles. po: 2 psum banks."""
                ysl = {}
                for kd in range(KT):
                    mm, c = kd // 8, kd % 8
                    yt = fypool.tile([128, 1024], MMDT, tag=f"ysl{kd}",
                                     name="ysl")
                    nc.sync.dma_start(
                        out=yt,
                        in_=yall[(b, mm)][c * 128:(c + 1) * 128,
                                          tsp * 1024:(tsp + 1) * 1024])
                    ysl[kd] = yt
                for ts in range(2):
                    po = [fpps.tile([128, 512], F32, tag=f"po{mo}",
                                    name=f"po{mo}") for mo in range(2)]
                    for kd in range(KT):
                        for mo in range(2):
                            nc.tensor.matmul(
                                po[mo], wo_sb[:, kd, mo * 128:(mo + 1) * 128],
                                ysl[kd][:, ts * 512:(ts + 1) * 512],
                                start=(kd == 0), stop=(kd == KT - 1))
                    for mo in range(2):
                        ot = fopool.tile([128, 512], F32, tag="ot", name="ot")
                        nc.vector.tensor_copy(ot, po[mo])
                        nc.sync.dma_start(
                            out=outT[mo * 128:(mo + 1) * 128,
                                     b * T + tsp * 1024 + ts * 512:
                                     b * T + tsp * 1024 + (ts + 1) * 512],
                            in_=ot)

            emit_pair(0, 0, {})
            emit_pair(0, 1, {})
            emit_pair(1, 0, {})
            emit_final_tsp(0, 0)
            emit_pair(1, 1, {1: lambda: emit_final_tsp(0, 1)})
            emit_final_tsp(1, 0)
            emit_final_tsp(1, 1)

    nc.compile()
    return nc


def _get_nc():
    global _CACHED_NC
    if _CACHED_NC is None:
        _CACHED_NC = _build_nc()
    return _CACHED_NC


def kernel(x, Wq, Wk, Wv, Wg, Wo, q_rms_w, k_rms_w, o_norm_w):
    global LAST_EXEC_TIME_NS
    import ml_dtypes
    npdt = ml_dtypes.bfloat16
    x = np.asarray(x, dtype=np.float32)
    Wq = np.asarray(Wq, dtype=np.float32)
    Wk = np.asarray(Wk, dtype=np.float32)
    Wv = np.asarray(Wv, dtype=np.float32)
    Wg = np.asarray(Wg, dtype=np.float32)
    Wo = np.asarray(Wo, dtype=np.float32)
    q_rms_w = np.asarray(q_rms_w, dtype=np.float32)
    k_rms_w = np.asarray(k_rms_w, dtype=np.float32)
    o_norm_w = np.asarray(o_norm_w, dtype=np.float32)

    xT = x.reshape(BT, HID).T          # [HID, BT]
    # [KT, NCH, 128, 1024] contiguous chunks
    xt4 = np.ascontiguousarray(
        xT.reshape(KT, 128, NCH, 1024).transpose(0, 2, 1, 3)).astype(npdt)
    # fold o_norm_w into Wo rows: (y*o_w) @ Wo == y @ (o_w[:,None]*Wo)
    wo_scaled = Wo * np.tile(o_norm_w, H)[:, None]

    inv = 1.0 / (10000.0 ** (np.arange(0, D, 2, dtype=np.float64) / D))
    pos = np.arange(T, dtype=np.float64)
    fr = pos[:, None] * inv[None, :]          # [T, 64]
    cosT = np.cos(fr).T.astype(np.float32)    # [64, T]
    sinT = np.sin(fr).T.astype(np.float32)
    cosbt = np.concatenate([cosT] * B, axis=1)
    sinbt = np.concatenate([sinT] * B, axis=1)
    cos2 = np.ascontiguousarray(np.vstack([cosbt, cosbt]))   # [128, BT]
    # rows 0-63 pre-negated: rope as stage*cos + swap(stage)*sin_signed
    sin2 = np.ascontiguousarray(np.vstack([-sinbt, sinbt]))

    kk, qq = np.meshgrid(np.arange(128), np.arange(128), indexing="ij")
    negm = np.where(kk <= qq, 0.0, NEG).astype(np.float32)
    ident = np.eye(128, dtype=np.float32)

    # wo rows permuted to the (m, core) AllGather-chunk kd order
    perm = np.concatenate(
        [np.arange(128) + (2 * cc + mm) * 128
         for mm in range(HC) for cc in range(NCORES)])
    wo_perm = wo_scaled[perm]

    in_maps = []
    for c in range(NCORES):
        csl = slice(c * DC, (c + 1) * DC)

        def wt(wmat):
            # [HID, DC] -> [128, KT, DC] matching the SBUF tile layout
            return np.ascontiguousarray(
                wmat[:, csl].reshape(KT, 128, DC).transpose(1, 0, 2)
            ).astype(npdt)

        in_maps.append({
            "xT": xt4,
            "wq": wt(Wq),
            "wk": wt(Wk),
            "wv": wt(Wv),
            "wg": wt(Wg),
            "wo": wt(wo_perm),
            "cos2": cos2,
            "sin2": sin2,
            "negm": negm,
            "ident_in": ident,
            "qrw": np.ascontiguousarray(q_rms_w.reshape(128, 1)),
            "krw": np.ascontiguousarray(k_rms_w.reshape(128, 1)),
        })

    nc = _get_nc()
    trace = os.environ.get("KERNEL_TRACE", "0") == "1"
    res = run_bass_kernel_spmd(nc, in_maps, list(range(NCORES)), trace=trace)
    LAST_EXEC_TIME_NS = res.exec_time_ns

    outT_full = np.concatenate([res.results[c]["outT"] for c in range(NCORES)],
                               axis=0)              # [2048 n, 4096 t]
    out = outT_full.T.reshape(B, T, HID)
    return np.ascontiguousarray(out)


# revision 6
# speedup vs baseline: 1.3762x; 1.3762x over previous
"""Trainium2 Bass kernel for a dense transformer block (QKV+gate proj, RoPE,
QK-RMSNorm, causal SDPA, output-RMSNorm + SiLU gate, output projection).

Sharding: tensor-parallel over heads across 8 NeuronCores (2 heads/core).
Wq/Wk/Wv/Wg split column-wise; attention fully local per core; per-(batch,
head) attention outputs are AllGathered in 4 chunks (overlapped with
compute) and the output projection uses Wo column-split (each core emits a
256-column slice of the output).

v3 design notes:
- RMS partition-sums on TensorE (ones matmul) with deferred issue: the ss
  matmuls, rsqrt factors and final scaled writes of chunk c are emitted in
  the middle of chunk c+1's matmul stream so the PE never waits on the
  scalar/vector evacuation chain.
- The softmax denominator is never computed: rms_norm(y/s) is evaluated
  as ystash * rsqrt(sum_d(ystash^2)/D) (the eps*s^2 term is negligible).
- ScalarE activation-table thrash eliminated: sweep A uses only
  {copy, square, abs_reciprocal_sqrt}; sweep B only {copy, silu};
  attention only {exp, ln} (rsqrt = exp(-0.5*ln(x))). 3 table loads.
- silu(gate) computed in sweep B's evacuation and kept in SBUF.
- V kept d-major after sweep B; transposed on TensorE in each attention
  pair's prologue (psum shared with the PV tag).
- AllGather split per (batch, head-chunk): 4 collectives that overlap
  attention; the final projection is interleaved into the attention
  instruction stream once its AG chunk has landed.
"""

import os
import sys

for _p in ("/opt/trn_rl_repo", "/root/.axon_site/_ro/trn_rl_repo"):
    if os.path.isdir(_p) and _p not in sys.path:
        sys.path.insert(0, _p)

import numpy as np

import concourse.mybir as mybir
from concourse import bacc
from concourse.bass_utils import run_bass_kernel_spmd
from concourse.tile import TileContext

B, T, HID = 2, 2048, 2048
H, D = 16, 128
NCORES = 8
HC = H // NCORES          # heads per core = 2
DC = HC * D               # 256 head-dims per core
BT = B * T                # 4096 tokens
KT = HID // 128           # 16 contraction tiles
EPS = 1e-5
SCALE = 1.0 / float(np.sqrt(D))
NEG = -3.0e38

F32 = mybir.dt.float32
BF16 = mybir.dt.bfloat16
AF = mybir.ActivationFunctionType
ALU = mybir.AluOpType

MMDT = BF16

LAST_EXEC_TIME_NS = None
_CACHED_NC = None


def _proj_sweep(nc, tc, xT, w_aps, posts, n_ss_tags):
    """One sweep over x computing 2 matrices (4 head-groups of 128) with
    512-token chunks.  posts[mi](m, ch, ps, tpool, pend) evacuates one tag;
    PE work and anything downstream of it must be appended to `pend`
    (closures), which is flushed a few matmuls into the NEXT chunk."""
    with tc.tile_pool(name="sweep_w", bufs=1) as wpool, \
         tc.tile_pool(name="sweep_x", bufs=4) as xpool, \
         tc.tile_pool(name="sweep_ps", bufs=1, space="PSUM") as pps, \
         tc.tile_pool(name="sweep_t", bufs=2) as tpool:
        wsb = []
        for mi, w_ap in enumerate(w_aps):
            w_t = wpool.tile([128, KT, DC], MMDT, tag=f"w{mi}", name=f"w{mi}")
            nc.sync.dma_start(out=w_t, in_=w_ap)
            wsb.append(w_t)
        # tags: (0,0),(0,1),(1,0) double-buffered; (1,1) single + ss bank
        tag_bufs = {(0, 0): 2, (0, 1): 2, (1, 0): 2, (1, 1): 1}
        pend = []
        xks = {}
        for ch in range(BT // 512):
            ps = {}
            for mi in range(2):
                for m in range(HC):
                    ps[(mi, m)] = pps.tile(
                        [128, 512], F32, tag=f"pp{mi}{m}",
                        name=f"pp{mi}{m}", bufs=tag_bufs[(mi, m)])
            if ch % 2 == 0:
                for k in range(KT):
                    xk = xpool.tile([128, 1024], MMDT, tag=f"xk{k}",
                                    name="xk", bufs=1)
                    nc.sync.dma_start(out=xk, in_=xT[k, ch // 2])
                    xks[k] = xk
            hsl = slice((ch % 2) * 512, (ch % 2) * 512 + 512)
            for k in range(KT):
                for mi in range(2):
                    for m in range(HC):
                        nc.tensor.matmul(
                            ps[(mi, m)], wsb[mi][:, k, m * 128:(m + 1) * 128],
                            xks[k][:, hsl], start=(k == 0),
                            stop=(k == KT - 1))
                if k == 5 and pend:
                    while pend:
                        pend.pop(0)()
            # evacuate in reverse order: single-buffered tag (1,1) first
            for mi in (1, 0):
                for m in (1, 0):
                    posts[mi](m, ch, ps[(mi, m)], tpool, pend)
        while pend:
            pend.pop(0)()


def _build_nc():
    nc = bacc.Bacc("TRN2", target_bir_lowering=False, debug=False,
                   num_devices=NCORES)

    xT = nc.dram_tensor("xT", [KT, BT // 1024, 128, 1024], MMDT,
                        kind="ExternalInput").ap()
    wq = nc.dram_tensor("wq", [128, KT, DC], MMDT, kind="ExternalInput").ap()
    wk = nc.dram_tensor("wk", [128, KT, DC], MMDT, kind="ExternalInput").ap()
    wv = nc.dram_tensor("wv", [128, KT, DC], MMDT, kind="ExternalInput").ap()
    wg = nc.dram_tensor("wg", [128, KT, DC], MMDT, kind="ExternalInput").ap()
    wo = nc.dram_tensor("wo", [128, KT, DC], MMDT, kind="ExternalInput").ap()
    cos2 = nc.dram_tensor("cos2", [128, BT], F32, kind="ExternalInput").ap()
    sin2 = nc.dram_tensor("sin2", [128, BT], F32, kind="ExternalInput").ap()
    negm = nc.dram_tensor("negm", [128, 128], F32, kind="ExternalInput").ap()
    ones_in = nc.dram_tensor("ones_in", [128, 128], MMDT,
                             kind="ExternalInput").ap()
    ident_in = nc.dram_tensor("ident_in", [128, 128], F32,
                              kind="ExternalInput").ap()
    qrw = nc.dram_tensor("qrw", [128, 1], F32, kind="ExternalInput").ap()
    krw = nc.dram_tensor("krw", [128, 1], F32, kind="ExternalInput").ap()

    outT = nc.dram_tensor("outT", [DC, BT], F32, kind="ExternalOutput").ap()
    ag_in = {}
    yall = {}
    for b in range(B):
        for m in range(HC):
            ag_in[(b, m)] = nc.dram_tensor(f"agin{b}{m}", [128, T],
                                           MMDT).ap()
            yall[(b, m)] = nc.dram_tensor(f"yall{b}{m}", [NCORES * 128, T],
                                          MMDT, addr_space="Shared").ap()

    with TileContext(nc) as tc:
      with tc.tile_pool(name="const", bufs=1) as const, \
           tc.tile_pool(name="persist", bufs=1) as persist:
        epsb = const.tile([128, 1], F32)
        nc.vector.memset(epsb, EPS)
        zerob = const.tile([128, 1], F32)
        nc.vector.memset(zerob, 0.0)
        ones_sb = const.tile([128, 128], MMDT)
        nc.sync.dma_start(out=ones_sb, in_=ones_in)

        # persistent per-head activations
        qTf = [persist.tile([128, BT], MMDT, tag=f"qTf{m}", name=f"qTf{m}")
               for m in range(HC)]
        kTf = [persist.tile([128, BT], MMDT, tag=f"kTf{m}", name=f"kTf{m}")
               for m in range(HC)]
        # v kept d-major in f32 (transposed per attention pair)
        v_dmaj = [persist.tile([128, BT], F32, tag=f"v{m}", name=f"v{m}")
                  for m in range(HC)]
        sg_sb = [persist.tile([128, BT], MMDT, tag=f"sg{m}", name=f"sg{m}")
                 for m in range(HC)]

        # ---------------- sweep A: q, k (rope + rms fused) ----------------
        with tc.tile_pool(name="ropec", bufs=1) as rp, \
             tc.tile_pool(name="ssps", bufs=1, space="PSUM") as ssps:
            cos_sb = rp.tile([128, BT], F32)
            sin_sb = rp.tile([128, BT], F32)
            qrw_sb = rp.tile([128, 1], F32)
            krw_sb = rp.tile([128, 1], F32)
            rope_loaded = [False]

            def load_rope_consts():
                # deferred so the first x/w DMAs win the queue
                nc.sync.dma_start(out=cos_sb, in_=cos2)
                nc.sync.dma_start(out=sin_sb, in_=sin2)
                nc.sync.dma_start(out=qrw_sb, in_=qrw)
                nc.sync.dma_start(out=krw_sb, in_=krw)

            def make_qk_post(dest, w_scalar):
                def post(m, ch, ps, tpool, pend):
                    if not rope_loaded[0]:
                        rope_loaded[0] = True
                        load_rope_consts()
                    c0, c1 = ch * 512, (ch + 1) * 512
                    stage = tpool.tile([128, 512], F32, tag="stage",
                                       name="stage", bufs=4)
                    nc.scalar.copy(stage, ps)
                    sq = tpool.tile([128, 512], MMDT, tag="sq", name="sq",
                                    bufs=4)
                    nc.vector.tensor_mul(sq, stage, stage)
                    # rope: sw = swap-halves(stage); sin_sb rows 0-63 are
                    # pre-negated so ro = stage*cos + sw*sin in one add
                    sw = tpool.tile([128, 512], F32, tag="sw", name="sw")
                    nc.sync.dma_start(out=sw[0:64, :], in_=stage[64:128, :])
                    nc.sync.dma_start(out=sw[64:128, :], in_=stage[0:64, :])
                    u = tpool.tile([128, 512], F32, tag="u", name="u")
                    nc.vector.tensor_mul(u, stage, cos_sb[:, c0:c1])
                    w = tpool.tile([128, 512], F32, tag="w", name="w")
                    nc.gpsimd.tensor_mul(w, sw, sin_sb[:, c0:c1])
                    ro = tpool.tile([128, 512], F32, tag="ro", name="ro")
                    nc.vector.tensor_add(ro, u, w)

                    def finish(m=m, ch=ch, stage=stage, sq=sq, ro=ro,
                               c0=c0, c1=c1):
                        # rms sum over head-dim via ones matmul (1 bank)
                        ssp = ssps.tile([128, 512], F32, tag="ss", name="ssp")
                        nc.tensor.matmul(ssp, ones_sb, sq,
                                         start=True, stop=True)
                        fac = tpool.tile([128, 512], F32, tag="fac",
                                         name="fac")
                        nc.scalar.activation(out=fac, in_=ssp,
                                             func=AF.Abs_reciprocal_sqrt,
                                             scale=1.0 / float(D), bias=epsb)
                        nc.vector.scalar_tensor_tensor(
                            out=dest[m][:, c0:c1], in0=ro, scalar=w_scalar,
                            in1=fac, op0=ALU.mult, op1=ALU.mult)
                    pend.append(finish)
                return post

            _proj_sweep(nc, tc, xT, [wq, wk],
                        [make_qk_post(qTf, qrw_sb),
                         make_qk_post(kTf, krw_sb)], 1)

        # ---------------- sweep B: v (d-major copy), gate (silu) ----------
        def v_post(m, ch, ps, tpool, pend):
            nc.scalar.copy(v_dmaj[m][:, ch * 512:(ch + 1) * 512], ps)

        def g_post(m, ch, ps, tpool, pend):
            nc.scalar.activation(
                out=sg_sb[m][:, ch * 512:(ch + 1) * 512], in_=ps,
                func=AF.Silu)

        _proj_sweep(nc, tc, xT, [wv, wg], [v_post, g_post], 0)

        # ---------------- attention + chunked AG + final ------------------
        with tc.tile_pool(name="at_ps", bufs=1, space="PSUM") as pps2, \
             tc.tile_pool(name="fin_ps", bufs=1, space="PSUM") as fpps, \
             tc.tile_pool(name="at_slab", bufs=1) as slab, \
             tc.tile_pool(name="at_t", bufs=2) as tpool, \
             tc.tile_pool(name="at_bh", bufs=1) as bhpool, \
             tc.tile_pool(name="fin_w", bufs=1) as fwpool, \
             tc.tile_pool(name="fin_y", bufs=1) as fypool, \
             tc.tile_pool(name="fin_o", bufs=2) as fopool, \
             tc.tile_pool(name="at_c", bufs=1) as acp:
            negm_sb = acp.tile([128, 128], F32)
            nc.sync.dma_start(out=negm_sb, in_=negm)
            ident = acp.tile([128, 128], F32)
            nc.sync.dma_start(out=ident, in_=ident_in)
            wo_sb = fwpool.tile([128, KT, DC], MMDT, tag="wo")
            nc.sync.dma_start(out=wo_sb, in_=wo)

            def emit_pair(b, m, fillers):
                """Attention for batch b, head-chunk m. fillers: dict
                qb -> closure emitted after that qb's PV block."""
                t0 = b * T
                # prologue: transpose this pair's V to token-major
                vT = slab.tile([128, 16, 128], MMDT, tag="vT", name="vT",
                               bufs=2)
                for s in range(4):
                    tp = pps2.tile([128, 512], F32, tag="yt", name="tp",
                                   bufs=2)
                    for j in range(4):
                        i = s * 4 + j
                        nc.tensor.transpose(
                            tp[:, j * 128:(j + 1) * 128],
                            v_dmaj[m][:, t0 + i * 128:t0 + (i + 1) * 128],
                            ident)
                    nc.vector.tensor_copy(vT[:, s * 4:(s + 1) * 4, :], tp)

                ystash = bhpool.tile([128, T], F32, tag="bhC", name="ystash")
                lnv = bhpool.tile([128, T], F32, tag="bhB", name="lnv")
                pend = []
                for qb in range(T // 512):
                    nk = 4 * (qb + 1)
                    es = []
                    for i2 in range(0, nk, 2):
                        stp = pps2.tile([128, 1024], F32, tag="st",
                                        name="stp", bufs=2)
                        e = slab.tile([128, 1024], MMDT, tag=f"es{i2 // 2}",
                                      name=f"es{i2 // 2}")
                        for j in range(2):
                            i = i2 + j
                            sl = slice(j * 512, (j + 1) * 512)
                            nc.tensor.matmul(
                                stp[:, sl],
                                kTf[m][:, t0 + i * 128:t0 + (i + 1) * 128],
                                qTf[m][:, t0 + qb * 512:t0 + (qb + 1) * 512],
                                start=True, stop=True)
                            q_off = i * 128 - qb * 512
                            if q_off >= 0:
                                nc.vector.tensor_add(
                                    stp[:, j * 512 + q_off:
                                        j * 512 + q_off + 128],
                                    stp[:, j * 512 + q_off:
                                        j * 512 + q_off + 128],
                                    negm_sb)
                                if j == 0 and q_off > 0:
                                    nc.gpsimd.memset(e[:, 0:q_off], 0.0)
                                if j == 1 and q_off > 0:
                                    nc.vector.memset(
                                        stp[:, 512:512 + q_off], NEG)
                        q_off0 = i2 * 128 - qb * 512
                        lo = max(0, q_off0)
                        nc.scalar.activation(
                            out=e[:, lo:], in_=stp[:, lo:],
                            func=AF.Exp, scale=SCALE)
                        es.append(e)
                        if i2 == 2 and pend:
                            while pend:
                                pend.pop(0)()
                    ytp = pps2.tile([128, 512], F32, tag="yt", name="ytp",
                                    bufs=2)
                    for i2 in range(0, nk, 2):
                        for j in range(2):
                            i = i2 + j
                            sl = slice(j * 512, (j + 1) * 512)
                            nc.tensor.matmul(
                                ytp, vT[:, i, :], es[i2 // 2][:, sl],
                                start=(i == 0), stop=(i == nk - 1))
                    qsl = slice(qb * 512, (qb + 1) * 512)
                    nc.vector.tensor_copy(ystash[:, qsl], ytp)
                    sqy = tpool.tile([128, 512], MMDT, tag="sqy", name="sqy")
                    nc.vector.tensor_mul(sqy, ystash[:, qsl], ystash[:, qsl])
                    ssyp = pps2.tile([128, 512], F32, tag="yt", name="ssyp",
                                     bufs=2)

                    def ssy_fin(qsl=qsl, sqy=sqy, ssyp=ssyp):
                        nc.tensor.matmul(ssyp, ones_sb, sqy,
                                         start=True, stop=True)
                        nc.scalar.activation(out=lnv[:, qsl], in_=ssyp,
                                             func=AF.Ln,
                                             scale=1.0 / float(D), bias=zerob)
                    pend.append(ssy_fin)
                    if qb in fillers:
                        fillers[qb]()
                while pend:
                    pend.pop(0)()
                # tail: fb2 = rsqrt(ssy/D) via exp(-0.5*ln); yf = ystash*fb2*sg
                fb2 = bhpool.tile([128, T], F32, tag="bhS", name="fb2")
                nc.scalar.activation(out=fb2, in_=lnv, func=AF.Exp,
                                     scale=-0.5, bias=zerob)
                yf1 = bhpool.tile([128, T], F32, tag="bhB", name="yf1")
                nc.vector.tensor_mul(yf1, ystash, fb2)
                yf = bhpool.tile([128, T], MMDT, tag="bhC", name="yf")
                nc.vector.tensor_mul(yf, yf1, sg_sb[m][:, t0:t0 + T])
                nc.sync.dma_start(out=ag_in[(b, m)], in_=yf)
                nc.gpsimd.collective_compute(
                    "AllGather", ALU.bypass,
                    ins=[ag_in[(b, m)]], outs=[yall[(b, m)]],
                    replica_groups=[list(range(NCORES))],
                )

            def emit_final_tsp(b, tsp):
                """Final projection for 1024 tokens (2 ts-slabs of 512) of
                batch b; contracts all 16 kd tiles. po: 2 psum banks."""
                ysl = {}
                for kd in range(KT):
                    mm, c = kd // 8, kd % 8
                    yt = fypool.tile([128, 1024], MMDT, tag=f"ysl{kd}",
                                     name="ysl")
                    nc.sync.dma_start(
                        out=yt,
                        in_=yall[(b, mm)][c * 128:(c + 1) * 128,
                                          tsp * 1024:(tsp + 1) * 1024])
                    ysl[kd] = yt
                for ts in range(2):
                    po = [fpps.tile([128, 512], F32, tag=f"po{mo}",
                                    name=f"po{mo}") for mo in range(2)]
                    for kd in range(KT):
                        for mo in range(2):
                            nc.tensor.matmul(
                                po[mo], wo_sb[:, kd, mo * 128:(mo + 1) * 128],
                                ysl[kd][:, ts * 512:(ts + 1) * 512],
                                start=(kd == 0), stop=(kd == KT - 1))
                    for mo in range(2):
                        ot = fopool.tile([128, 512], F32, tag="ot", name="ot")
                        nc.vector.tensor_copy(ot, po[mo])
                        nc.sync.dma_start(
                            out=outT[mo * 128:(mo + 1) * 128,
                                     b * T + tsp * 1024 + ts * 512:
                                     b * T + tsp * 1024 + (ts + 1) * 512],
                            in_=ot)

            emit_pair(0, 0, {})
            emit_pair(0, 1, {})
            emit_pair(1, 0, {})
            emit_final_tsp(0, 0)
            emit_pair(1, 1, {1: lambda: emit_final_tsp(0, 1)})
            emit_final_tsp(1, 0)
            emit_final_tsp(1, 1)

    nc.compile()
    return nc


def _get_nc():
    global _CACHED_NC
    if _CACHED_NC is None:
        _CACHED_NC = _build_nc()
    return _CACHED_NC


def kernel(x, Wq, Wk, Wv, Wg, Wo, q_rms_w, k_rms_w, o_norm_w):
    global LAST_EXEC_TIME_NS
    import ml_dtypes
    npdt = ml_dtypes.bfloat16
    x = np.asarray(x, dtype=np.float32)
    Wq = np.asarray(Wq, dtype=np.float32)
    Wk = np.asarray(Wk, dtype=np.float32)
    Wv = np.asarray(Wv, dtype=np.float32)
    Wg = np.asarray(Wg, dtype=np.float32)
    Wo = np.asarray(Wo, dtype=np.float32)
    q_rms_w = np.asarray(q_rms_w, dtype=np.float32)
    k_rms_w = np.asarray(k_rms_w, dtype=np.float32)
    o_norm_w = np.asarray(o_norm_w, dtype=np.float32)

    xT = x.reshape(BT, HID).T          # [HID, BT]
    # [KT, BT//1024, 128, 1024] contiguous chunks
    xt4 = np.ascontiguousarray(
        xT.reshape(KT, 128, BT // 1024, 1024).transpose(0, 2, 1, 3)
    ).astype(npdt)
    # fold o_norm_w into Wo rows: (y*o_w) @ Wo == y @ (o_w[:,None]*Wo)
    wo_scaled = Wo * np.tile(o_norm_w, H)[:, None]

    inv = 1.0 / (10000.0 ** (np.arange(0, D, 2, dtype=np.float64) / D))
    pos = np.arange(T, dtype=np.float64)
    fr = pos[:, None] * inv[None, :]          # [T, 64]
    cosT = np.cos(fr).T.astype(np.float32)    # [64, T]
    sinT = np.sin(fr).T.astype(np.float32)
    cosbt = np.concatenate([cosT] * B, axis=1)
    sinbt = np.concatenate([sinT] * B, axis=1)
    cos2 = np.ascontiguousarray(np.vstack([cosbt, cosbt]))   # [128, BT]
    # rows 0-63 pre-negated: rope as stage*cos + swap(stage)*sin_signed
    sin2 = np.ascontiguousarray(np.vstack([-sinbt, sinbt]))

    kk, qq = np.meshgrid(np.arange(128), np.arange(128), indexing="ij")
    negm = np.where(kk <= qq, 0.0, NEG).astype(np.float32)
    ones128 = np.ones((128, 128), dtype=np.float32)
    ident = np.eye(128, dtype=np.float32)

    # wo rows permuted to the (m, core) AllGather-chunk kd order
    perm = np.concatenate(
        [np.arange(128) + (2 * cc + mm) * 128
         for mm in range(HC) for cc in range(NCORES)])
    wo_perm = wo_scaled[perm]

    in_maps = []
    for c in range(NCORES):
        csl = slice(c * DC, (c + 1) * DC)

        def wt(wmat):
            # [HID, DC] -> [128, KT, DC] matching the SBUF tile layout
            return np.ascontiguousarray(
                wmat[:, csl].reshape(KT, 128, DC).transpose(1, 0, 2)
            ).astype(npdt)

        in_maps.append({
            "xT": xt4,
            "wq": wt(Wq),
            "wk": wt(Wk),
            "wv": wt(Wv),
            "wg": wt(Wg),
            "wo": wt(wo_perm),
            "cos2": cos2,
            "sin2": sin2,
            "negm": negm,
            "ones_in": ones128.astype(npdt),
            "ident_in": ident,
            "qrw": np.ascontiguousarray(q_rms_w.reshape(128, 1)),
            "krw": np.ascontiguousarray(k_rms_w.reshape(128, 1)),
        })

    nc = _get_nc()
    trace = os.environ.get("KERNEL_TRACE", "0") == "1"
    res = run_bass_kernel_spmd(nc, in_maps, list(range(NCORES)), trace=trace)
    LAST_EXEC_TIME_NS = res.exec_time_ns

    outT_full = np.concatenate([res.results[c]["outT"] for c in range(NCORES)],
                               axis=0)              # [2048 n, 4096 t]
    out = outT_full.T.reshape(B, T, HID)
    return np.ascontiguousarray(out)


# revision 13
# speedup vs baseline: 1.4658x; 1.0651x over previous
"""Trainium2 Bass kernel for a dense transformer block (QKV+gate proj, RoPE,
QK-RMSNorm, causal SDPA, output-RMSNorm + SiLU gate, output projection).

Sharding: tensor-parallel over heads across 8 NeuronCores (2 heads/core).
Wq/Wk/Wv/Wg split column-wise; attention fully local per core; per-(batch,
head) attention outputs are AllGathered in 4 chunks (overlapped with
compute) and the output projection uses Wo column-split (each core emits a
256-column slice of the output).

v4 design notes:
- RMS partition-sums on TensorE (ones matmul) with deferred issue: the ss
  matmuls / rsqrt factors / final writes of chunk c are emitted mid-way
  through chunk c+1's matmul stream so the PE never waits on the
  scalar/vector evacuation chain.
- Softmax denominator never computed: rms_norm(y/s) == ystash *
  rsqrt(sum_d(ystash^2)/D) (the eps*s^2 term is negligible).
- Activation-table switches minimized: sweep A uses {copy,
  abs_reciprocal_sqrt} only; sweep B {copy, silu}; attention {exp} with
  one abs_reciprocal_sqrt call per pair (2 switches/pair).
- x tiles are shared between the sweeps: sweep B processes the last
  chunk-pair first, reusing sweep A's resident x tiles; wv/wg/wo DMAs are
  prefetched via hooks so no PE gap at phase transitions.
- V kept d-major; transposed on TensorE in each pair's prologue.
- AllGather split per (batch, head-chunk); the final projection for b0 is
  interleaved into the attention stream; final b1 runs m0-half first so
  its matmuls overlap the last AllGather.
"""

import os
import sys

for _p in ("/opt/trn_rl_repo", "/root/.axon_site/_ro/trn_rl_repo"):
    if os.path.isdir(_p) and _p not in sys.path:
        sys.path.insert(0, _p)

import numpy as np

import concourse.mybir as mybir
from concourse import bacc
from concourse.bass_utils import run_bass_kernel_spmd
from concourse.tile import TileContext

B, T, HID = 2, 2048, 2048
H, D = 16, 128
NCORES = 8
HC = H // NCORES          # heads per core = 2
DC = HC * D               # 256 head-dims per core
BT = B * T                # 4096 tokens
KT = HID // 128           # 16 contraction tiles
NCHK = BT // 512          # 8 projection chunks of 512 tokens
EPS = 1e-5
SCALE = 1.0 / float(np.sqrt(D))
NEG = -3.0e38

F32 = mybir.dt.float32
BF16 = mybir.dt.bfloat16
AF = mybir.ActivationFunctionType
ALU = mybir.AluOpType

MMDT = BF16

LAST_EXEC_TIME_NS = None
_CACHED_NC = None


def _proj_sweep(nc, tc, xpool, xT, wsb, posts, hooks, ch_order, xks,
                skip_fetch):
    """One sweep over x computing 2 matrices (4 head-groups of 128) with
    512-token chunks.  posts[mi](m, ch, ps, tpool, pend) evacuates one tag;
    TensorE work and anything downstream must go through `pend` (closures),
    flushed a few matmuls into the NEXT chunk.  x tiles live in the shared
    `xks` dict keyed by k; chunk-pairs listed in skip_fetch reuse resident
    tiles."""
    with tc.tile_pool(name="sweep_ps", bufs=1, space="PSUM") as pps, \
         tc.tile_pool(name="sweep_t", bufs=2) as tpool:
        tag_bufs = {(0, 0): 2, (0, 1): 2, (1, 0): 2, (1, 1): 1}
        pend = []
        for ci, ch in enumerate(ch_order):
            if ci in hooks:
                hooks[ci]()
            ps = {}
            for mi in range(2):
                for m in range(HC):
                    ps[(mi, m)] = pps.tile(
                        [128, 512], F32, tag=f"pp{mi}{m}",
                        name=f"pp{mi}{m}", bufs=tag_bufs[(mi, m)])
            if ch % 2 == 0 and ch // 2 not in skip_fetch:
                for k in range(KT):
                    xk = xpool.tile([128, 1024], MMDT, tag=f"xk{k}",
                                    name="xk", bufs=1)
                    nc.sync.dma_start(out=xk, in_=xT[k, ch // 2])
                    xks[k] = xk
            hsl = slice((ch % 2) * 512, (ch % 2) * 512 + 512)
            for k in range(KT):
                for mi in range(2):
                    for m in range(HC):
                        nc.tensor.matmul(
                            ps[(mi, m)], wsb[mi][:, k, m * 128:(m + 1) * 128],
                            xks[k][:, hsl], start=(k == 0),
                            stop=(k == KT - 1))
                if k == 5 and pend:
                    while pend:
                        pend.pop(0)()
            # evacuate in reverse order: single-buffered tag (1,1) first
            for mi in (1, 0):
                for m in (1, 0):
                    posts[mi](m, ch, ps[(mi, m)], tpool, pend)
        while pend:
            pend.pop(0)()


def _build_nc():
    nc = bacc.Bacc("TRN2", target_bir_lowering=False, debug=False,
                   num_devices=NCORES)

    xT = nc.dram_tensor("xT", [KT, BT // 1024, 128, 1024], MMDT,
                        kind="ExternalInput").ap()
    wq = nc.dram_tensor("wq", [128, KT, DC], MMDT, kind="ExternalInput").ap()
    wk = nc.dram_tensor("wk", [128, KT, DC], MMDT, kind="ExternalInput").ap()
    wv = nc.dram_tensor("wv", [128, KT, DC], MMDT, kind="ExternalInput").ap()
    wg = nc.dram_tensor("wg", [128, KT, DC], MMDT, kind="ExternalInput").ap()
    wo = nc.dram_tensor("wo", [128, KT, DC], MMDT, kind="ExternalInput").ap()
    cos2 = nc.dram_tensor("cos2", [128, T], F32, kind="ExternalInput").ap()
    sin2 = nc.dram_tensor("sin2", [128, T], F32, kind="ExternalInput").ap()
    negm = nc.dram_tensor("negm", [128, 128], F32, kind="ExternalInput").ap()
    ones_in = nc.dram_tensor("ones_in", [128, 128], MMDT,
                             kind="ExternalInput").ap()
    ident_in = nc.dram_tensor("ident_in", [128, 128], F32,
                              kind="ExternalInput").ap()
    qrw = nc.dram_tensor("qrw", [128, 1], F32, kind="ExternalInput").ap()
    krw = nc.dram_tensor("krw", [128, 1], F32, kind="ExternalInput").ap()

    outT = nc.dram_tensor("outT", [DC, BT], F32, kind="ExternalOutput").ap()
    ag_in = {}
    yall = {}
    for b in range(B):
        for m in range(HC):
            ag_in[(b, m)] = nc.dram_tensor(f"agin{b}{m}", [128, T],
                                           MMDT).ap()
            yall[(b, m)] = nc.dram_tensor(f"yall{b}{m}", [NCORES * 128, T],
                                          MMDT, addr_space="Shared").ap()

    with TileContext(nc) as tc:
      with tc.tile_pool(name="const", bufs=1) as const, \
           tc.tile_pool(name="persist", bufs=1) as persist, \
           tc.tile_pool(name="weightsB", bufs=1) as wpool, \
           tc.tile_pool(name="at_c", bufs=1) as acp:
        negm_sb = acp.tile([128, 128], F32)
        ident = acp.tile([128, 128], F32)
        wpoolA = tc.alloc_tile_pool(name="weightsA", bufs=1)
        xpool = tc.alloc_tile_pool(name="xpool", bufs=1)
        epsb = const.tile([128, 1], F32)
        nc.vector.memset(epsb, EPS)
        zerob = const.tile([128, 1], F32)
        nc.vector.memset(zerob, 0.0)
        ones_sb = const.tile([128, 128], MMDT)
        nc.sync.dma_start(out=ones_sb, in_=ones_in)

        # weight slabs; wq/wk DMA'd now, wv/wg/wo prefetched via hooks
        wq_sb = wpoolA.tile([128, KT, DC], MMDT, tag="wq")
        nc.sync.dma_start(out=wq_sb, in_=wq)
        wk_sb = wpoolA.tile([128, KT, DC], MMDT, tag="wk")
        nc.sync.dma_start(out=wk_sb, in_=wk)
        wv_sb = wpool.tile([128, KT, DC], MMDT, tag="wv")
        wg_sb = wpool.tile([128, KT, DC], MMDT, tag="wg")
        wo_sb = wpool.tile([128, KT, DC], MMDT, tag="wo")

        # persistent per-head activations
        qTf = [persist.tile([128, BT], MMDT, tag=f"qTf{m}", name=f"qTf{m}")
               for m in range(HC)]
        kTf = [persist.tile([128, BT], MMDT, tag=f"kTf{m}", name=f"kTf{m}")
               for m in range(HC)]
        # v kept d-major in f32 (transposed per attention pair)
        v_dmaj = [persist.tile([128, BT], F32, tag=f"v{m}", name=f"v{m}")
                  for m in range(HC)]
        sg_sb = [persist.tile([128, BT], MMDT, tag=f"sg{m}", name=f"sg{m}")
                 for m in range(HC)]

        xks = {}

        # ---------------- sweep A: q, k (rope + rms fused) ----------------
        with tc.tile_pool(name="ropec", bufs=1) as rp, \
             tc.tile_pool(name="ssps", bufs=1, space="PSUM") as ssps:
            cos_sb = rp.tile([128, T], F32)
            sin_sb = rp.tile([128, T], F32)
            qrw_sb = rp.tile([128, 1], F32)
            krw_sb = rp.tile([128, 1], F32)
            rope_loaded = [False]

            def load_rope_consts():
                # deferred so the first x/w DMAs win the queue
                nc.sync.dma_start(out=cos_sb, in_=cos2)
                nc.sync.dma_start(out=sin_sb, in_=sin2)
                nc.sync.dma_start(out=qrw_sb, in_=qrw)
                nc.sync.dma_start(out=krw_sb, in_=krw)

            def make_qk_post(dest, w_scalar):
                def post(m, ch, ps, tpool, pend):
                    if not rope_loaded[0]:
                        rope_loaded[0] = True
                        load_rope_consts()
                    c0, c1 = ch * 512, (ch + 1) * 512
                    r0, r1 = (ch % 4) * 512, (ch % 4) * 512 + 512
                    stage = tpool.tile([128, 512], F32, tag="stage",
                                       name="stage", bufs=3)
                    nc.scalar.copy(stage, ps)
                    sq = tpool.tile([128, 512], MMDT, tag="sq", name="sq",
                                    bufs=4)
                    nc.vector.tensor_mul(sq, stage, stage)
                    # rope: sw = swap-halves(stage); sin_sb rows 0-63 are
                    # pre-negated so ro = stage*cos + sw*sin in one add
                    sw = tpool.tile([128, 512], F32, tag="sw", name="sw")
                    nc.sync.dma_start(out=sw[0:64, :], in_=stage[64:128, :])
                    nc.sync.dma_start(out=sw[64:128, :], in_=stage[0:64, :])
                    u = tpool.tile([128, 512], F32, tag="u", name="u")
                    nc.vector.tensor_mul(u, stage, cos_sb[:, r0:r1])
                    w = tpool.tile([128, 512], F32, tag="w", name="w")
                    nc.gpsimd.tensor_mul(w, sw, sin_sb[:, r0:r1])
                    ro = tpool.tile([128, 512], F32, tag="ro", name="ro")
                    nc.vector.tensor_add(ro, u, w)

                    def finish(m=m, ch=ch, sq=sq, ro=ro, c0=c0, c1=c1):
                        # rms sum over head-dim via ones matmul (1 bank)
                        ssp = ssps.tile([128, 512], F32, tag="ss", name="ssp")
                        nc.tensor.matmul(ssp, ones_sb, sq,
                                         start=True, stop=True)
                        fac = tpool.tile([128, 512], F32, tag="fac",
                                         name="fac")
                        nc.scalar.activation(out=fac, in_=ssp,
                                             func=AF.Abs_reciprocal_sqrt,
                                             scale=1.0 / float(D), bias=epsb)
                        nc.vector.scalar_tensor_tensor(
                            out=dest[m][:, c0:c1], in0=ro, scalar=w_scalar,
                            in1=fac, op0=ALU.mult, op1=ALU.mult)
                    pend.append(finish)
                return post

            def hook_wvg():
                nc.sync.dma_start(out=wv_sb, in_=wv)
                nc.sync.dma_start(out=wg_sb, in_=wg)

            _proj_sweep(nc, tc, xpool, xT, [wq_sb, wk_sb],
                        [make_qk_post(qTf, qrw_sb),
                         make_qk_post(kTf, krw_sb)],
                        {5: hook_wvg}, list(range(NCHK)), xks, set())

        # ---------------- sweep B: v (d-major copy), gate (silu) ----------
        # processes the last chunk-pair first, reusing sweep A's resident
        # x tiles, then pairs 0..2
        if True:

            def v_post(m, ch, ps, tpool, pend):
                nc.scalar.copy(v_dmaj[m][:, ch * 512:(ch + 1) * 512], ps)

            def g_post(m, ch, ps, tpool, pend):
                nc.scalar.activation(
                    out=sg_sb[m][:, ch * 512:(ch + 1) * 512], in_=ps,
                    func=AF.Silu)

            def hook_wo():
                nc.sync.dma_start(out=wo_sb, in_=wo)
                nc.sync.dma_start(out=negm_sb, in_=negm)
                nc.sync.dma_start(out=ident, in_=ident_in)

            _proj_sweep(nc, tc, xpool, xT, [wv_sb, wg_sb], [v_post, g_post],
                        {3: hook_wo}, [6, 7, 0, 1, 2, 3, 4, 5], xks,
                        {3})
            xpool.release()
            wpoolA.release()

            # ------------- attention + chunked AG + final -----------------
            fopool = tc.alloc_tile_pool(name="fin_o", bufs=2)
            with tc.tile_pool(name="at_ps", bufs=1, space="PSUM") as pps2, \
                 tc.tile_pool(name="fin_ps", bufs=1, space="PSUM") as fpps, \
                 tc.tile_pool(name="at_slab", bufs=1) as slab, \
                 tc.tile_pool(name="at_t", bufs=2) as tpool, \
                 tc.tile_pool(name="at_bh", bufs=1) as bhpool:

                def emit_pair(b, m, fillers):
                    """Attention for batch b, head-chunk m. fillers: dict
                    qb -> closure emitted after that qb's PV block."""
                    t0 = b * T
                    # prologue: transpose this pair's V to token-major
                    vT = slab.tile([128, 16, 128], MMDT, tag="vT", name="vT",
                                   bufs=2)
                    for s in range(4):
                        tp = pps2.tile([128, 512], F32, tag="yt", name="tp",
                                       bufs=2)
                        for j in range(4):
                            i = s * 4 + j
                            nc.tensor.transpose(
                                tp[:, j * 128:(j + 1) * 128],
                                v_dmaj[m][:, t0 + i * 128:t0 + (i + 1) * 128],
                                ident)
                        nc.vector.tensor_copy(vT[:, s * 4:(s + 1) * 4, :], tp)

                    ystash = bhpool.tile([128, T], MMDT, tag="bhC",
                                         name="ystash")
                    ssy = bhpool.tile([128, T], MMDT, tag="bhS", name="ssy")
                    pend = []
                    for qb in range(T // 512):
                        nk = 4 * (qb + 1)
                        es = []
                        for i2 in range(0, nk, 2):
                            stp = pps2.tile([128, 1024], F32, tag="st",
                                            name="stp", bufs=2)
                            e = slab.tile([128, 1024], MMDT,
                                          tag=f"es{i2 // 2}",
                                          name=f"es{i2 // 2}")
                            for j in range(2):
                                i = i2 + j
                                sl = slice(j * 512, (j + 1) * 512)
                                nc.tensor.matmul(
                                    stp[:, sl],
                                    kTf[m][:, t0 + i * 128:
                                           t0 + (i + 1) * 128],
                                    qTf[m][:, t0 + qb * 512:
                                           t0 + (qb + 1) * 512],
                                    start=True, stop=True)
                                q_off = i * 128 - qb * 512
                                if q_off >= 0:
                                    nc.vector.tensor_add(
                                        stp[:, j * 512 + q_off:
                                            j * 512 + q_off + 128],
                                        stp[:, j * 512 + q_off:
                                            j * 512 + q_off + 128],
                                        negm_sb)
                                    if j == 0 and q_off > 0:
                                        nc.gpsimd.memset(e[:, 0:q_off], 0.0)
                                    if j == 1 and q_off > 0:
                                        nc.vector.memset(
                                            stp[:, 512:512 + q_off], NEG)
                            q_off0 = i2 * 128 - qb * 512
                            lo = max(0, q_off0)
                            nc.scalar.activation(
                                out=e[:, lo:], in_=stp[:, lo:],
                                func=AF.Exp, scale=SCALE)
                            es.append(e)
                            if i2 == 2 and pend:
                                while pend:
                                    pend.pop(0)()
                        ytp = pps2.tile([128, 512], F32, tag="yt",
                                        name="ytp", bufs=2)
                        for i2 in range(0, nk, 2):
                            for j in range(2):
                                i = i2 + j
                                sl = slice(j * 512, (j + 1) * 512)
                                nc.tensor.matmul(
                                    ytp, vT[:, i, :], es[i2 // 2][:, sl],
                                    start=(i == 0), stop=(i == nk - 1))
                        qsl = slice(qb * 512, (qb + 1) * 512)
                        nc.vector.tensor_copy(ystash[:, qsl], ytp)
                        sqy = tpool.tile([128, 512], MMDT, tag="sqy",
                                         name="sqy")
                        nc.vector.tensor_mul(sqy, ystash[:, qsl],
                                             ystash[:, qsl])
                        ssyp = pps2.tile([128, 512], F32, tag="yt",
                                         name="ssyp", bufs=2)

                        def ssy_fin(qsl=qsl, sqy=sqy, ssyp=ssyp):
                            nc.tensor.matmul(ssyp, ones_sb, sqy,
                                             start=True, stop=True)
                            nc.vector.tensor_copy(ssy[:, qsl], ssyp)
                        pend.append(ssy_fin)
                        if qb in fillers:
                            fillers[qb]()
                    while pend:
                        pend.pop(0)()
                    # tail: fb2 = rsqrt(ssy/D); yf = ystash*fb2*sg
                    fb2 = bhpool.tile([128, T], MMDT, tag="bhB", name="fb2")
                    nc.scalar.activation(out=fb2, in_=ssy,
                                         func=AF.Abs_reciprocal_sqrt,
                                         scale=1.0 / float(D), bias=zerob)
                    yf1 = bhpool.tile([128, T], MMDT, tag="bhS", name="yf1")
                    nc.vector.tensor_mul(yf1, ystash, fb2)
                    yf = bhpool.tile([128, T], MMDT, tag="bhC", name="yf")
                    nc.vector.tensor_mul(yf, yf1, sg_sb[m][:, t0:t0 + T])
                    nc.sync.dma_start(out=ag_in[(b, m)], in_=yf)
                    nc.gpsimd.collective_compute(
                        "AllGather", ALU.bypass,
                        ins=[ag_in[(b, m)]], outs=[yall[(b, m)]],
                        replica_groups=[list(range(NCORES))],
                    )

                def fetch_ysl(pool, b, tsp, kds, tagfn):
                    ysl = {}
                    for kd in kds:
                        mm, c = kd // 8, kd % 8
                        yt = pool.tile([128, 1024], MMDT, tag=tagfn(kd),
                                       name="ysl")
                        nc.sync.dma_start(
                            out=yt,
                            in_=yall[(b, mm)][c * 128:(c + 1) * 128,
                                              tsp * 1024:(tsp + 1) * 1024])
                        ysl[kd] = yt
                    return ysl

                def final_evac(b, tsp, ts, po):
                    for mo in range(2):
                        ot = fopool.tile([128, 512], F32, tag="ot",
                                         name="ot")
                        nc.vector.tensor_copy(ot, po[mo])
                        nc.sync.dma_start(
                            out=outT[mo * 128:(mo + 1) * 128,
                                     b * T + tsp * 1024 + ts * 512:
                                     b * T + tsp * 1024 + (ts + 1) * 512],
                            in_=ot)

                with tc.tile_pool(name="fin_y", bufs=1) as fypool:

                    def emit_final_tsp(b, tsp):
                        """Final projection for 1024 tokens of batch b;
                        contracts all 16 kd tiles. po: 2 psum banks."""
                        ysl = fetch_ysl(fypool, b, tsp, range(KT),
                                        lambda kd: f"ysl{kd}")
                        for ts in range(2):
                            po = [fpps.tile([128, 512], F32, tag=f"po{mo}",
                                            name=f"po{mo}")
                                  for mo in range(2)]
                            for kd in range(KT):
                                for mo in range(2):
                                    nc.tensor.matmul(
                                        po[mo],
                                        wo_sb[:, kd, mo * 128:(mo + 1) * 128],
                                        ysl[kd][:, ts * 512:(ts + 1) * 512],
                                        start=(kd == 0), stop=(kd == KT - 1))
                            final_evac(b, tsp, ts, po)

                    emit_pair(0, 0, {})
                    emit_pair(0, 1, {})
                    emit_pair(1, 0, {})
                    emit_final_tsp(0, 0)
                    emit_pair(1, 1, {1: lambda: emit_final_tsp(0, 1)})

            # ---- final b1: m0-half kds first (overlap the last AG) ----
            # attention psum pools are released; use 8 held accumulators
            with tc.tile_pool(name="fin2_ps", bufs=1, space="PSUM") as f2ps, \
                 tc.tile_pool(name="fin_y2", bufs=2) as fypool2:
                ysl_m0 = {tsp: fetch_ysl(fypool2, 1, tsp, range(8),
                                         lambda kd: f"y2_{kd % 8}")
                          for tsp in range(2)}
                pos = {}
                for tsp in range(2):
                    for ts in range(2):
                        po = [f2ps.tile([128, 512], F32,
                                        tag=f"po{tsp}{ts}{mo}",
                                        name="po") for mo in range(2)]
                        pos[(tsp, ts)] = po
                        for kd in range(8):
                            for mo in range(2):
                                nc.tensor.matmul(
                                    po[mo],
                                    wo_sb[:, kd, mo * 128:(mo + 1) * 128],
                                    ysl_m0[tsp][kd][:,
                                                    ts * 512:(ts + 1) * 512],
                                    start=(kd == 0), stop=False)
                ysl_m1 = {tsp: fetch_ysl(fypool2, 1, tsp, range(8, KT),
                                         lambda kd: f"y2_{kd % 8}")
                          for tsp in range(2)}
                for tsp in range(2):
                    for ts in range(2):
                        po = pos[(tsp, ts)]
                        for kd in range(8, KT):
                            for mo in range(2):
                                nc.tensor.matmul(
                                    po[mo],
                                    wo_sb[:, kd, mo * 128:(mo + 1) * 128],
                                    ysl_m1[tsp][kd][:,
                                                    ts * 512:(ts + 1) * 512],
                                    start=False, stop=(kd == KT - 1))
                        final_evac(1, tsp, ts, po)
            fopool.release()

    nc.compile()
    return nc


def _get_nc():
    global _CACHED_NC
    if _CACHED_NC is None:
        _CACHED_NC = _build_nc()
    return _CACHED_NC


def kernel(x, Wq, Wk, Wv, Wg, Wo, q_rms_w, k_rms_w, o_norm_w):
    global LAST_EXEC_TIME_NS
    import ml_dtypes
    npdt = ml_dtypes.bfloat16
    x = np.asarray(x, dtype=np.float32)
    Wq = np.asarray(Wq, dtype=np.float32)
    Wk = np.asarray(Wk, dtype=np.float32)
    Wv = np.asarray(Wv, dtype=np.float32)
    Wg = np.asarray(Wg, dtype=np.float32)
    Wo = np.asarray(Wo, dtype=np.float32)
    q_rms_w = np.asarray(q_rms_w, dtype=np.float32)
    k_rms_w = np.asarray(k_rms_w, dtype=np.float32)
    o_norm_w = np.asarray(o_norm_w, dtype=np.float32)

    xT = x.reshape(BT, HID).T          # [HID, BT]
    # [KT, BT//1024, 128, 1024] contiguous chunks
    xt4 = np.ascontiguousarray(
        xT.reshape(KT, 128, BT // 1024, 1024).transpose(0, 2, 1, 3)
    ).astype(npdt)
    # fold o_norm_w into Wo rows: (y*o_w) @ Wo == y @ (o_w[:,None]*Wo)
    wo_scaled = Wo * np.tile(o_norm_w, H)[:, None]

    inv = 1.0 / (10000.0 ** (np.arange(0, D, 2, dtype=np.float64) / D))
    pos = np.arange(T, dtype=np.float64)
    fr = pos[:, None] * inv[None, :]          # [T, 64]
    cosT = np.cos(fr).T.astype(np.float32)    # [64, T]
    sinT = np.sin(fr).T.astype(np.float32)
    cos2 = np.ascontiguousarray(np.vstack([cosT, cosT]))     # [128, T]
    # rows 0-63 pre-negated: rope as stage*cos + swap(stage)*sin_signed
    sin2 = np.ascontiguousarray(np.vstack([-sinT, sinT]))

    kk, qq = np.meshgrid(np.arange(128), np.arange(128), indexing="ij")
    negm = np.where(kk <= qq, 0.0, NEG).astype(np.float32)
    ones128 = np.ones((128, 128), dtype=np.float32)
    ident = np.eye(128, dtype=np.float32)

    # wo rows permuted to the (m, core) AllGather-chunk kd order
    perm = np.concatenate(
        [np.arange(128) + (2 * cc + mm) * 128
         for mm in range(HC) for cc in range(NCORES)])
    wo_perm = wo_scaled[perm]

    in_maps = []
    for c in range(NCORES):
        csl = slice(c * DC, (c + 1) * DC)

        def wt(wmat):
            # [HID, DC] -> [128, KT, DC] matching the SBUF tile layout
            return np.ascontiguousarray(
                wmat[:, csl].reshape(KT, 128, DC).transpose(1, 0, 2)
            ).astype(npdt)

        in_maps.append({
            "xT": xt4,
            "wq": wt(Wq),
            "wk": wt(Wk),
            "wv": wt(Wv),
            "wg": wt(Wg),
            "wo": wt(wo_perm),
            "cos2": cos2,
            "sin2": sin2,
            "negm": negm,
            "ones_in": ones128.astype(npdt),
            "ident_in": ident,
            "qrw": np.ascontiguousarray(q_rms_w.reshape(128, 1)),
            "krw": np.ascontiguousarray(k_rms_w.reshape(128, 1)),
        })

    nc = _get_nc()
    trace = os.environ.get("KERNEL_TRACE", "0") == "1"
    res = run_bass_kernel_spmd(nc, in_maps, list(range(NCORES)), trace=trace)
    LAST_EXEC_TIME_NS = res.exec_time_ns

    outT_full = np.concatenate([res.results[c]["outT"] for c in range(NCORES)],
                               axis=0)              # [2048 n, 4096 t]
    out = outT_full.T.reshape(B, T, HID)
    return np.ascontiguousarray(out)
